# revision 6
# baseline (speedup 1.0000x reference)
import numpy as np

B, CIN, H, W = 2, 16, 64, 64
COUT, P = 64, 3
K = 3
I_TOT = CIN * K * K
N_CORES = 8
ROWS_PER_CORE = 16
N_LOC = ROWS_PER_CORE * W

_STATE = {}


def _install_prof_shim():
    try:
        import sys, types

        if "antenv.axon_hooks" not in sys.modules:
            mod = types.ModuleType("antenv.axon_hooks")
            holder = [None]
            mod.set_axon_ntff_profile_hook = lambda h: holder.__setitem__(0, h)
            mod.get_axon_ntff_profile_hook = lambda: holder[0]
            sys.modules["antenv.axon_hooks"] = mod
            import antenv

            antenv.axon_hooks = mod
            try:
                from trn_agent_boot.trn_boot import _ntff_profile_via_ctypes

                hook = _ntff_profile_via_ctypes("/opt/axon/libaxon_pjrt.so")
                mod.set_axon_ntff_profile_hook(hook)
            except Exception:
                pass
        import concourse.bass_utils as bu

        if getattr(bu.upload_artifacts, "__name__", "") != "<lambda>":
            bu.upload_artifacts = lambda tmpdir: tmpdir
    except Exception:
        pass


def _build_program():
    import concourse.mybir as mybir
    from concourse import bacc
    from concourse.tile import TileContext

    f32 = mybir.dt.float32
    sub = mybir.AluOpType.subtract
    mult = mybir.AluOpType.mult
    div = mybir.AluOpType.divide
    mx = mybir.AluOpType.max
    add = mybir.AluOpType.add

    nc = bacc.Bacc("TRN2", target_bir_lowering=False, num_devices=N_CORES)
    x_d = nc.dram_tensor("x", [CIN, ROWS_PER_CORE + 2, W + 2], f32, kind="ExternalInput")
    pos_d = nc.dram_tensor("pos", [I_TOT, COUT * P], f32, kind="ExternalInput")
    val_d = nc.dram_tensor("val", [I_TOT, COUT * P], f32, kind="ExternalInput")
    out_d = nc.dram_tensor("out", [COUT, N_LOC], f32, kind="ExternalOutput")

    with TileContext(nc) as tc:
        with (
            tc.tile_pool(name="cp", bufs=1) as cp,
            tc.tile_pool(name="pp", bufs=2, space="PSUM") as pp,
            tc.tile_pool(name="bp", bufs=1, space="PSUM") as bp,
        ):
            pos0 = cp.tile([128, COUT * P], f32)
            val0 = cp.tile([128, COUT * P], f32)
            pos1 = cp.tile([16, COUT * P], f32)
            val1 = cp.tile([16, COUT * P], f32)
            nc.sync.dma_start(out=pos0[:], in_=pos_d[0:128, :])
            nc.sync.dma_start(out=val0[:], in_=val_d[0:128, :])
            nc.sync.dma_start(out=pos1[:], in_=pos_d[128:I_TOT, :])
            nc.sync.dma_start(out=val1[:], in_=val_d[128:I_TOT, :])

            xsh = cp.tile([CIN, ROWS_PER_CORE + 2, W + 2], f32)
            nc.sync.dma_start(out=xsh[:], in_=x_d[:])

            def table_math(pos_t, val_t, n_p, a_all, wd, b1):
                p3 = pos_t[:].rearrange("p (o k) -> p o k", k=P)
                v3 = val_t[:].rearrange("p (o k) -> p o k", k=P)
                d_all = cp.tile([n_p, COUT, 2], f32, tag=f"d{n_p}")
                dv_all = cp.tile([n_p, COUT, 2], f32, tag=f"dv{n_p}")
                tmp = cp.tile([n_p, COUT], f32, tag=f"tm{n_p}")
                nc.vector.tensor_tensor(d_all[:], p3[:, :, 1:3], p3[:, :, 0:2], sub)
                nc.vector.tensor_tensor(dv_all[:], v3[:, :, 1:3], v3[:, :, 0:2], sub)
                nc.vector.reciprocal(d_all[:], d_all[:])
                nc.vector.tensor_tensor(a_all[:], dv_all[:], d_all[:], mult)
                nc.vector.tensor_tensor(wd[:], a_all[:, :, 1], a_all[:, :, 0], sub)
                nc.vector.tensor_tensor(tmp[:], p3[:, :, 1], a_all[:, :, 0], mult)
                nc.vector.tensor_tensor(b1[:], v3[:, :, 1], tmp[:], sub)

            a_all0 = cp.tile([128, COUT, 2], f32)
            wd0 = cp.tile([128, COUT], f32)
            b10 = cp.tile([128, COUT], f32)
            table_math(pos0, val0, 128, a_all0, wd0, b10)

            a_all1 = cp.tile([16, COUT, 2], f32)
            wd1 = cp.tile([16, COUT], f32)
            b11 = cp.tile([16, COUT], f32)
            table_math(pos1, val1, 16, a_all1, wd1, b11)

            wtail = cp.tile([48, COUT], f32)
            nc.vector.memset(wtail[:], 0.0)
            nc.sync.dma_start(
                out=wtail[0:16, :], in_=a_all1[:, :, 0]
            )
            nc.sync.dma_start(out=wtail[32:48, :], in_=wd1[:])

            ones0 = cp.tile([128, 1], f32)
            ones1 = cp.tile([16, 1], f32)
            nc.vector.memset(ones0[:], 1.0)
            nc.vector.memset(ones1[:], 1.0)
            ps_b = bp.tile([COUT, 1], f32)
            nc.tensor.matmul(ps_b[:], b10[:], ones0[:], start=True, stop=False)
            nc.tensor.matmul(ps_b[:], b11[:], ones1[:], start=False, stop=True)
            bias = cp.tile([COUT, 1], f32)
            nc.vector.tensor_copy(bias[:], ps_b[:])

            p1_tail = cp.tile([48, 1], f32)
            nc.sync.dma_start(out=p1_tail[32:48, :], in_=pos1[:, 1:2])

            xt0 = cp.tile([128, N_LOC], f32)
            xp1 = cp.tile([48, N_LOC], f32)
            for w in range(9):
                kh, kw = divmod(w, 3)
                srcw = xsh[:, kh : kh + ROWS_PER_CORE, kw : kw + W]
                if w < 8:
                    dst = xt0[w * 16 : (w + 1) * 16, :].rearrange(
                        "p (a b) -> p a b", a=ROWS_PER_CORE, b=W
                    )
                    nc.sync.dma_start(out=dst, in_=srcw)
                else:
                    for pbase in (0, 16, 32):
                        dst = xp1[pbase : pbase + 16, :].rearrange(
                            "p (a b) -> p a b", a=ROWS_PER_CORE, b=W
                        )
                        nc.sync.dma_start(out=dst, in_=srcw)

            pos_t = cp.tile([128, N_LOC], f32)
            nc.vector.tensor_scalar(pos_t[:], xt0[:], pos0[:, 1:2], 0.0, sub, mx)
            nc.vector.tensor_scalar(
                xp1[32:48, :], xp1[32:48, :], p1_tail[32:48, :], 0.0, sub, mx
            )

            ob = cp.tile([COUT, N_LOC], f32)
            n_t = 512
            for nt in range(N_LOC // n_t):
                sl = slice(nt * n_t, (nt + 1) * n_t)
                ps = pp.tile([COUT, n_t], f32, tag="ps")
                nc.tensor.matmul(ps[:], a_all0[:, :, 0], xt0[:, sl], start=True, stop=False)
                nc.tensor.matmul(ps[:], wd0[:], pos_t[:, sl], start=False, stop=False)
                nc.tensor.matmul(ps[:], wtail[:], xp1[:, sl], start=False, stop=True)
                nc.vector.tensor_scalar(ob[:, sl], ps[:], bias[:], None, add)

            nc.sync.dma_start(out=out_d[:], in_=ob[:])

    nc.compile()
    return nc


def _fast_path_ok(positions):
    if positions.shape != (I_TOT, COUT, P):
        return False
    p = positions
    if not np.all(p[:, :, 1] == p[:, 0:1, 1]):
        return False
    if not (np.all(p[:, :, 0] < p[:, :, 1]) and np.all(p[:, :, 1] < p[:, :, 2])):
        return False
    return True


def _reference_numpy(x, positions, values):
    xf = x.astype(np.float32)
    Bs, C, Hs, Ws = xf.shape
    xp = np.pad(xf, ((0, 0), (0, 0), (1, 1), (1, 1)))
    cols = [xp[:, :, i : i + Hs, j : j + Ws] for i in range(K) for j in range(K)]
    pch = np.stack(cols, 2).reshape(Bs, C * K * K, Hs * Ws)
    X = pch.transpose(0, 2, 1).reshape(-1, C * K * K)
    Np, Ii = X.shape
    Pp = positions.shape[-1]
    out = np.zeros((Np, positions.shape[1]), np.float32)
    chunk = 1024
    for st in range(0, Np, chunk):
        xb = X[st : st + chunk, :, None]
        idx = np.sum(xb[..., None] >= positions[None], axis=-1)
        idx = np.clip(idx, 1, Pp - 1)
        f = np.zeros((xb.shape[0], Ii, positions.shape[1]), np.float32)
        for s in range(1, Pp):
            x0 = positions[:, :, s - 1]
            x1 = positions[:, :, s]
            y0 = values[:, :, s - 1]
            y1 = values[:, :, s]
            t = (xb - x0) / (x1 - x0)
            f = np.where(idx == s, y0 + t * (y1 - y0), f)
        out[st : st + chunk] = f.sum(axis=1)
    O = out.shape[-1]
    return out.reshape(Bs, Hs * Ws, O).transpose(0, 2, 1).reshape(Bs, O, Hs, Ws)


def kernel(x, positions, values):
    x = np.ascontiguousarray(x, dtype=np.float32)
    positions = np.ascontiguousarray(positions, dtype=np.float32)
    values = np.ascontiguousarray(values, dtype=np.float32)

    if not _fast_path_ok(positions):
        return _reference_numpy(x, positions, values)

    _install_prof_shim()
    from concourse.bass_utils import run_bass_kernel_spmd

    if "nc" not in _STATE:
        _STATE["nc"] = _build_program()
    nc = _STATE["nc"]

    pos_r = np.ascontiguousarray(
        positions.reshape(CIN, K * K, COUT, P).transpose(1, 0, 2, 3)
    ).reshape(I_TOT, COUT * P)
    val_r = np.ascontiguousarray(
        values.reshape(CIN, K * K, COUT, P).transpose(1, 0, 2, 3)
    ).reshape(I_TOT, COUT * P)

    xp = np.pad(x, ((0, 0), (0, 0), (1, 1), (1, 1)))
    in_maps = []
    for k in range(N_CORES):
        b, y0 = divmod(k, N_CORES // B)
        y0 *= ROWS_PER_CORE
        xs = np.ascontiguousarray(xp[b, :, y0 : y0 + ROWS_PER_CORE + 2, :])
        in_maps.append({"x": xs, "pos": pos_r, "val": val_r})

    res = run_bass_kernel_spmd(nc, in_maps, core_ids=list(range(N_CORES)))
    _STATE["last_result"] = res

    out = np.empty((B, COUT, H, W), np.float32)
    for k in range(N_CORES):
        b, y0 = divmod(k, N_CORES // B)
        y0 *= ROWS_PER_CORE
        out[b, :, y0 : y0 + ROWS_PER_CORE, :] = res.results[k]["out"].reshape(
            COUT, ROWS_PER_CORE, W
        )
    return out


# revision 8
# speedup vs baseline: 1.3970x; 1.3970x over previous
import numpy as np

B, CIN, H, W = 2, 16, 64, 64
COUT, P = 64, 3
K = 3
I_TOT = CIN * K * K
N_CORES = 8
ROWS_PER_CORE = 16
N_LOC = ROWS_PER_CORE * W
KA = 2 * K * CIN
KB = K * CIN

_STATE = {}


def _install_prof_shim():
    try:
        import sys, types

        if "antenv.axon_hooks" not in sys.modules:
            mod = types.ModuleType("antenv.axon_hooks")
            holder = [None]
            mod.set_axon_ntff_profile_hook = lambda h: holder.__setitem__(0, h)
            mod.get_axon_ntff_profile_hook = lambda: holder[0]
            sys.modules["antenv.axon_hooks"] = mod
            import antenv

            antenv.axon_hooks = mod
            try:
                from trn_agent_boot.trn_boot import _ntff_profile_via_ctypes

                hook = _ntff_profile_via_ctypes("/opt/axon/libaxon_pjrt.so")
                mod.set_axon_ntff_profile_hook(hook)
            except Exception:
                pass
        import concourse.bass_utils as bu

        if getattr(bu.upload_artifacts, "__name__", "") != "<lambda>":
            bu.upload_artifacts = lambda tmpdir: tmpdir
    except Exception:
        pass


def _set_ap(ap, dims, offset):
    v = ap.ap
    v.clear()
    for d in dims:
        v.append(list(d))
    ap.offset = offset
    return ap


def _build_program():
    import concourse.mybir as mybir
    from concourse import bacc
    from concourse.tile import TileContext

    f32 = mybir.dt.float32
    bf16 = mybir.dt.bfloat16
    sub = mybir.AluOpType.subtract
    mult = mybir.AluOpType.mult
    mx = mybir.AluOpType.max
    add = mybir.AluOpType.add
    act_id = mybir.ActivationFunctionType.Identity

    nc = bacc.Bacc("TRN2", target_bir_lowering=False, num_devices=N_CORES)
    x_d = nc.dram_tensor(
        "x3", [CIN, K, ROWS_PER_CORE + 2, W], f32, kind="ExternalInput"
    )
    tbl_d = nc.dram_tensor("tbl", [I_TOT, 2 * COUT * P], f32, kind="ExternalInput")
    out_d = nc.dram_tensor("out", [COUT, N_LOC], f32, kind="ExternalOutput")

    with TileContext(nc) as tc:
        with (
            tc.tile_pool(name="cp", bufs=1) as cp,
            tc.tile_pool(name="pp", bufs=1, space="PSUM") as pp,
        ):
            tblA = cp.tile([KA, 2 * COUT * P], f32)
            tblB = cp.tile([KB, 2 * COUT * P], f32)
            nc.scalar.dma_start(out=tblA[:], in_=tbl_d[0:KA, :])
            nc.scalar.dma_start(out=tblB[:], in_=tbl_d[KA:I_TOT, :])

            xbf = cp.tile([CIN, K, ROWS_PER_CORE + 2, W], bf16)
            nc.gpsimd.dma_start(out=xbf[:], in_=x_d[:])

            xta = cp.tile([KA, N_LOC], bf16)
            xtb = cp.tile([KB, N_LOC], bf16)
            nc.sync.dma_start(
                out=xta[0:KB, :], in_=xbf[:, :, 0:ROWS_PER_CORE, :]
            )
            nc.sync.dma_start(
                out=xta[KB:KA, :], in_=xbf[:, :, 1 : 1 + ROWS_PER_CORE, :]
            )
            nc.scalar.dma_start(
                out=xtb[:], in_=xbf[:, :, 2 : 2 + ROWS_PER_CORE, :]
            )

            def table_math(tbl_t, n_p, sfx):
                p3 = tbl_t[:, 0 : COUT * P].rearrange("p (o k) -> p o k", k=P)
                v3 = tbl_t[:, COUT * P :].rearrange("p (o k) -> p o k", k=P)
                d_all = cp.tile([n_p, COUT, 2], f32, tag=f"d{sfx}")
                dv_all = cp.tile([n_p, COUT, 2], f32, tag=f"dv{sfx}")
                a_all = cp.tile([n_p, COUT, 2], f32, tag=f"a{sfx}")
                tmp = cp.tile([n_p, COUT], f32, tag=f"t{sfx}")
                w1 = cp.tile([n_p, COUT], bf16, tag=f"w1{sfx}")
                wd = cp.tile([n_p, COUT], bf16, tag=f"wd{sfx}")
                b1 = cp.tile([n_p, COUT], bf16, tag=f"b1{sfx}")
                p1 = cp.tile([n_p, 1], f32, tag=f"p1{sfx}")
                nc.vector.tensor_tensor(d_all[:], p3[:, :, 1:3], p3[:, :, 0:2], sub)
                nc.vector.reciprocal_approx_fast(d_all[:], d_all[:])
                nc.vector.tensor_tensor(dv_all[:], v3[:, :, 1:3], v3[:, :, 0:2], sub)
                nc.vector.tensor_tensor(a_all[:], dv_all[:], d_all[:], mult)
                nc.vector.tensor_copy(w1[:], a_all[:, :, 0])
                nc.vector.tensor_tensor(wd[:], a_all[:, :, 1], a_all[:, :, 0], sub)
                nc.vector.tensor_tensor(tmp[:], p3[:, :, 1], a_all[:, :, 0], mult)
                nc.vector.tensor_tensor(b1[:], v3[:, :, 1], tmp[:], sub)
                nc.vector.tensor_copy(p1[:], tbl_t[:, 1:2])
                return w1, wd, b1, p1

            w1A, wdA, b1A, p1A = table_math(tblA, KA, "A")
            w1B, wdB, b1B, p1B = table_math(tblB, KB, "B")

            posA = cp.tile([KA, N_LOC], bf16)
            posB = cp.tile([KB, N_LOC], bf16)
            nc.vector.tensor_scalar(posA[:], xta[:], p1A[:], 0.0, sub, mx)
            nc.vector.tensor_scalar(posB[:], xtb[:], p1B[:], 0.0, sub, mx)

            onesA = cp.tile([KA, 1], bf16)
            onesB = cp.tile([KB, 1], bf16)
            nc.vector.memset(onesA[:], 1.0)
            nc.vector.memset(onesB[:], 1.0)
            psb = pp.tile([128, 1], f32, tag="psb")
            for cg in (0, COUT):
                tp = (0, cg)
                sl = slice(cg, cg + COUT)
                nc.tensor.matmul(
                    psb[sl, :], b1A[:], onesA[:], start=True, stop=False, tile_position=tp
                )
                nc.tensor.matmul(
                    psb[sl, :], b1B[:], onesB[:], start=False, stop=True, tile_position=tp
                )
            bias = cp.tile([128, 1], f32)
            nc.vector.tensor_copy(bias[:], psb[:])

            ps = pp.tile([128, 512], f32, tag="ps")
            for cg in (0, COUT):
                tp = (0, cg)
                osl = slice(cg, cg + COUT)
                nsl = slice((cg // COUT) * 512, (cg // COUT) * 512 + 512)
                nc.tensor.matmul(
                    ps[osl, :], w1A[:], xta[:, nsl], start=True, stop=False,
                    tile_position=tp,
                )
                nc.tensor.matmul(
                    ps[osl, :], wdA[:], posA[:, nsl], start=False, stop=False,
                    tile_position=tp,
                )
                nc.tensor.matmul(
                    ps[osl, :], w1B[:], xtb[:, nsl], start=False, stop=False,
                    tile_position=tp,
                )
                nc.tensor.matmul(
                    ps[osl, :], wdB[:], posB[:, nsl], start=False, stop=True,
                    tile_position=tp,
                )

            ob = cp.tile([128, 512], f32)
            nc.scalar.activation(ob[:], ps[:], act_id, bias=bias[:])
            dst = _set_ap(
                out_d.ap().copy(), [[512, 2], [N_LOC, COUT], [1, 512]], 0
            )
            nc.sync.dma_start(out=dst, in_=ob[:])

    nc.compile()
    return nc


def _fast_path_ok(positions):
    if positions.shape != (I_TOT, COUT, P):
        return False
    p = positions
    if not np.all(p[:, :, 1] == p[:, 0:1, 1]):
        return False
    if not (np.all(p[:, :, 0] < p[:, :, 1]) and np.all(p[:, :, 1] < p[:, :, 2])):
        return False
    return True


def _reference_numpy(x, positions, values):
    xf = x.astype(np.float32)
    Bs, C, Hs, Ws = xf.shape
    xp = np.pad(xf, ((0, 0), (0, 0), (1, 1), (1, 1)))
    cols = [xp[:, :, i : i + Hs, j : j + Ws] for i in range(K) for j in range(K)]
    pch = np.stack(cols, 2).reshape(Bs, C * K * K, Hs * Ws)
    X = pch.transpose(0, 2, 1).reshape(-1, C * K * K)
    Np, Ii = X.shape
    Pp = positions.shape[-1]
    out = np.zeros((Np, positions.shape[1]), np.float32)
    chunk = 1024
    for st in range(0, Np, chunk):
        xb = X[st : st + chunk, :, None]
        idx = np.sum(xb[..., None] >= positions[None], axis=-1)
        idx = np.clip(idx, 1, Pp - 1)
        f = np.zeros((xb.shape[0], Ii, positions.shape[1]), np.float32)
        for s in range(1, Pp):
            x0 = positions[:, :, s - 1]
            x1 = positions[:, :, s]
            y0 = values[:, :, s - 1]
            y1 = values[:, :, s]
            t = (xb - x0) / (x1 - x0)
            f = np.where(idx == s, y0 + t * (y1 - y0), f)
        out[st : st + chunk] = f.sum(axis=1)
    O = out.shape[-1]
    return out.reshape(Bs, Hs * Ws, O).transpose(0, 2, 1).reshape(Bs, O, Hs, Ws)


def kernel(x, positions, values):
    x = np.ascontiguousarray(x, dtype=np.float32)
    positions = np.ascontiguousarray(positions, dtype=np.float32)
    values = np.ascontiguousarray(values, dtype=np.float32)

    if not _fast_path_ok(positions):
        return _reference_numpy(x, positions, values)

    _install_prof_shim()
    from concourse.bass_utils import run_bass_kernel_spmd

    if "nc" not in _STATE:
        _STATE["nc"] = _build_program()
    nc = _STATE["nc"]

    pos_r = positions.reshape(CIN, K, K, COUT * P).transpose(1, 0, 2, 3)
    val_r = values.reshape(CIN, K, K, COUT * P).transpose(1, 0, 2, 3)
    tbl = np.concatenate(
        [pos_r.reshape(I_TOT, COUT * P), val_r.reshape(I_TOT, COUT * P)], axis=1
    )
    tbl = np.ascontiguousarray(tbl)

    xp = np.pad(x, ((0, 0), (0, 0), (1, 1), (1, 1)))
    in_maps = []
    for k in range(N_CORES):
        b, y0 = divmod(k, N_CORES // B)
        y0 *= ROWS_PER_CORE
        slab = xp[b, :, y0 : y0 + ROWS_PER_CORE + 2, :]
        x3 = np.empty((CIN, K, ROWS_PER_CORE + 2, W), np.float32)
        for kw in range(K):
            x3[:, kw] = slab[:, :, kw : kw + W]
        in_maps.append({"x3": x3, "tbl": tbl})

    res = run_bass_kernel_spmd(nc, in_maps, core_ids=list(range(N_CORES)))
    _STATE["last_result"] = res

    out = np.empty((B, COUT, H, W), np.float32)
    for k in range(N_CORES):
        b, y0 = divmod(k, N_CORES // B)
        y0 *= ROWS_PER_CORE
        out[b, :, y0 : y0 + ROWS_PER_CORE, :] = res.results[k]["out"].reshape(
            COUT, ROWS_PER_CORE, W
        )
    return out


# revision 9
# speedup vs baseline: 1.5286x; 1.0942x over previous
import numpy as np

B, CIN, H, W = 2, 16, 64, 64
COUT, P = 64, 3
K = 3
I_TOT = CIN * K * K
N_CORES = 8
ROWS_PER_CORE = 16
N_LOC = ROWS_PER_CORE * W
KCH = K * CIN
XFREE = (ROWS_PER_CORE + 2) * W

_STATE = {}


def _install_prof_shim():
    try:
        import sys, types

        if "antenv.axon_hooks" not in sys.modules:
            mod = types.ModuleType("antenv.axon_hooks")
            holder = [None]
            mod.set_axon_ntff_profile_hook = lambda h: holder.__setitem__(0, h)
            mod.get_axon_ntff_profile_hook = lambda: holder[0]
            sys.modules["antenv.axon_hooks"] = mod
            import antenv

            antenv.axon_hooks = mod
            try:
                from trn_agent_boot.trn_boot import _ntff_profile_via_ctypes

                hook = _ntff_profile_via_ctypes("/opt/axon/libaxon_pjrt.so")
                mod.set_axon_ntff_profile_hook(hook)
            except Exception:
                pass
        import concourse.bass_utils as bu

        if getattr(bu.upload_artifacts, "__name__", "") != "<lambda>":
            bu.upload_artifacts = lambda tmpdir: tmpdir
    except Exception:
        pass


def _set_ap(ap, dims, offset):
    v = ap.ap
    v.clear()
    for d in dims:
        v.append(list(d))
    ap.offset = offset
    return ap


def _build_program():
    import concourse.mybir as mybir
    from concourse import bacc
    from concourse.tile import TileContext

    f32 = mybir.dt.float32
    bf16 = mybir.dt.bfloat16
    sub = mybir.AluOpType.subtract
    mult = mybir.AluOpType.mult
    mx = mybir.AluOpType.max
    act_id = mybir.ActivationFunctionType.Identity

    nc = bacc.Bacc("TRN2", target_bir_lowering=False, num_devices=N_CORES)
    x_d = nc.dram_tensor("x3b", [KCH, XFREE], f32, kind="ExternalInput")
    tbl_d = nc.dram_tensor("tbl", [I_TOT, 2 * COUT * P], f32, kind="ExternalInput")
    out_d = nc.dram_tensor("out", [COUT, N_LOC], f32, kind="ExternalOutput")

    with TileContext(nc) as tc:
        with (
            tc.tile_pool(name="cp", bufs=1) as cp,
            tc.tile_pool(name="pp", bufs=1, space="PSUM") as pp,
        ):
            xf = cp.tile([KCH, XFREE], f32)
            nc.sync.dma_start(out=xf[:], in_=x_d[:])
            tblA = cp.tile([2 * KCH, 2 * COUT * P], f32)
            tblB = cp.tile([KCH, 2 * COUT * P], f32)
            nc.scalar.dma_start(out=tblA[:], in_=tbl_d[0 : 2 * KCH, :])
            nc.scalar.dma_start(out=tblB[:], in_=tbl_d[2 * KCH : I_TOT, :])

            xbf = cp.tile([KCH, XFREE], bf16)
            nc.vector.tensor_copy(xbf[:], xf[:])
            pos3 = cp.tile([KCH, XFREE], bf16)
            nc.vector.tensor_scalar(pos3[:], xbf[:], tblB[:, 1:2], 0.0, sub, mx)

            def table_math(tbl_t, n_p, sfx):
                p3 = tbl_t[:, 0 : COUT * P].rearrange("p (o k) -> p o k", k=P)
                v3 = tbl_t[:, COUT * P :].rearrange("p (o k) -> p o k", k=P)
                d_all = cp.tile([n_p, COUT, 2], f32, tag=f"d{sfx}")
                dv_all = cp.tile([n_p, COUT, 2], f32, tag=f"dv{sfx}")
                a_all = cp.tile([n_p, COUT, 2], f32, tag=f"a{sfx}")
                w1 = cp.tile([n_p, COUT], bf16, tag=f"w1{sfx}")
                wd = cp.tile([n_p, COUT], bf16, tag=f"wd{sfx}")
                nc.vector.tensor_tensor(d_all[:], p3[:, :, 1:3], p3[:, :, 0:2], sub)
                nc.vector.reciprocal_approx_fast(d_all[:], d_all[:])
                nc.vector.tensor_tensor(dv_all[:], v3[:, :, 1:3], v3[:, :, 0:2], sub)
                nc.vector.tensor_tensor(a_all[:], dv_all[:], d_all[:], mult)
                nc.vector.tensor_copy(w1[:], a_all[:, :, 0])
                nc.vector.tensor_tensor(wd[:], a_all[:, :, 1], a_all[:, :, 0], sub)
                return p3, v3, a_all, w1, wd

            p3A, v3A, aA, w1A, wdA = table_math(tblA, 2 * KCH, "A")
            p3B, v3B, aB, w1B, wdB = table_math(tblB, KCH, "B")

            w1s = cp.tile([KCH, COUT], bf16)
            wds = cp.tile([KCH, COUT], bf16)
            nc.scalar.dma_start(out=w1s[:], in_=w1A[KCH : 2 * KCH, :])
            nc.scalar.dma_start(out=wds[:], in_=wdA[KCH : 2 * KCH, :])

            ps = pp.tile([128, 512], f32, tag="ps")
            lhs_seq = [
                (w1A[0:KCH, :], xbf, 0),
                (w1B[:], xbf, 2),
                (wdA[0:KCH, :], pos3, 0),
                (wdB[:], pos3, 2),
                (w1s[:], xbf, 1),
                (wds[:], pos3, 1),
            ]
            for si, (wt, rhs_t, kh) in enumerate(lhs_seq):
                for cg in (0, COUT):
                    nsl = slice(kh * W + (cg // COUT) * 512, kh * W + (cg // COUT) * 512 + 512)
                    nc.tensor.matmul(
                        ps[cg : cg + COUT, :],
                        wt,
                        rhs_t[:, nsl],
                        start=(si == 0),
                        stop=(si == len(lhs_seq) - 1),
                        tile_position=(0, cg),
                    )

            tmpA = cp.tile([2 * KCH, COUT], f32)
            b1A = cp.tile([2 * KCH, COUT], bf16)
            nc.vector.tensor_tensor(tmpA[:], p3A[:, :, 1], aA[:, :, 0], mult)
            nc.vector.tensor_tensor(b1A[:], v3A[:, :, 1], tmpA[:], sub)
            tmpB = cp.tile([KCH, COUT], f32)
            b1B = cp.tile([KCH, COUT], bf16)
            nc.vector.tensor_tensor(tmpB[:], p3B[:, :, 1], aB[:, :, 0], mult)
            nc.vector.tensor_tensor(b1B[:], v3B[:, :, 1], tmpB[:], sub)
            onesA = cp.tile([2 * KCH, 1], bf16)
            onesB = cp.tile([KCH, 1], bf16)
            nc.vector.memset(onesA[:], 1.0)
            nc.vector.memset(onesB[:], 1.0)
            psb = pp.tile([128, 1], f32, tag="psb")
            for cg in (0, COUT):
                nc.tensor.matmul(
                    psb[cg : cg + COUT, :], b1A[:], onesA[:],
                    start=True, stop=False, tile_position=(0, cg),
                )
                nc.tensor.matmul(
                    psb[cg : cg + COUT, :], b1B[:], onesB[:],
                    start=False, stop=True, tile_position=(0, cg),
                )
            bias = cp.tile([128, 1], f32)
            nc.vector.tensor_copy(bias[:], psb[:])

            ob = cp.tile([128, 512], f32)
            nc.scalar.activation(ob[:], ps[:], act_id, bias=bias[:])
            dst = _set_ap(out_d.ap().copy(), [[512, 2], [N_LOC, COUT], [1, 512]], 0)
            nc.sync.dma_start(out=dst, in_=ob[:])

    nc.compile()
    return nc


def _fast_path_ok(positions):
    if positions.shape != (I_TOT, COUT, P):
        return False
    p = positions
    if np.ptp(p[:, :, 1]) != 0.0:
        return False
    if not (np.all(p[:, :, 0] < p[:, :, 1]) and np.all(p[:, :, 1] < p[:, :, 2])):
        return False
    return True


def _reference_numpy(x, positions, values):
    xf = x.astype(np.float32)
    Bs, C, Hs, Ws = xf.shape
    xp = np.pad(xf, ((0, 0), (0, 0), (1, 1), (1, 1)))
    cols = [xp[:, :, i : i + Hs, j : j + Ws] for i in range(K) for j in range(K)]
    pch = np.stack(cols, 2).reshape(Bs, C * K * K, Hs * Ws)
    X = pch.transpose(0, 2, 1).reshape(-1, C * K * K)
    Np, Ii = X.shape
    Pp = positions.shape[-1]
    out = np.zeros((Np, positions.shape[1]), np.float32)
    chunk = 1024
    for st in range(0, Np, chunk):
        xb = X[st : st + chunk, :, None]
        idx = np.sum(xb[..., None] >= positions[None], axis=-1)
        idx = np.clip(idx, 1, Pp - 1)
        f = np.zeros((xb.shape[0], Ii, positions.shape[1]), np.float32)
        for s in range(1, Pp):
            x0 = positions[:, :, s - 1]
            x1 = positions[:, :, s]
            y0 = values[:, :, s - 1]
            y1 = values[:, :, s]
            t = (xb - x0) / (x1 - x0)
            f = np.where(idx == s, y0 + t * (y1 - y0), f)
        out[st : st + chunk] = f.sum(axis=1)
    O = out.shape[-1]
    return out.reshape(Bs, Hs * Ws, O).transpose(0, 2, 1).reshape(Bs, O, Hs, Ws)


def kernel(x, positions, values):
    x = np.ascontiguousarray(x, dtype=np.float32)
    positions = np.ascontiguousarray(positions, dtype=np.float32)
    values = np.ascontiguousarray(values, dtype=np.float32)

    if not _fast_path_ok(positions):
        return _reference_numpy(x, positions, values)

    _install_prof_shim()
    from concourse.bass_utils import run_bass_kernel_spmd

    if "nc" not in _STATE:
        _STATE["nc"] = _build_program()
    nc = _STATE["nc"]

    pos_r = positions.reshape(CIN, K, K, COUT * P).transpose(1, 0, 2, 3)
    val_r = values.reshape(CIN, K, K, COUT * P).transpose(1, 0, 2, 3)
    tbl = np.concatenate(
        [pos_r.reshape(I_TOT, COUT * P), val_r.reshape(I_TOT, COUT * P)], axis=1
    )
    tbl = np.ascontiguousarray(tbl)

    xp = np.pad(x, ((0, 0), (0, 0), (1, 1), (1, 1)))
    in_maps = []
    for k in range(N_CORES):
        b, y0 = divmod(k, N_CORES // B)
        y0 *= ROWS_PER_CORE
        slab = xp[b, :, y0 : y0 + ROWS_PER_CORE + 2, :]
        x3 = np.empty((CIN, K, ROWS_PER_CORE + 2, W), np.float32)
        for kw in range(K):
            x3[:, kw] = slab[:, :, kw : kw + W]
        in_maps.append({"x3b": x3.reshape(KCH, XFREE), "tbl": tbl})

    res = run_bass_kernel_spmd(nc, in_maps, core_ids=list(range(N_CORES)))
    _STATE["last_result"] = res

    out = np.empty((B, COUT, H, W), np.float32)
    for k in range(N_CORES):
        b, y0 = divmod(k, N_CORES // B)
        y0 *= ROWS_PER_CORE
        out[b, :, y0 : y0 + ROWS_PER_CORE, :] = res.results[k]["out"].reshape(
            COUT, ROWS_PER_CORE, W
        )
    return out
